# revision 39
# baseline (speedup 1.0000x reference)
"""Trainium2 Bass kernel for ComiRec dynamic-routing (CapsNet-style) layer.

Problem: B=1024, S=200, E=128, C=128, n_caps=4, 3 routing rounds.

Sharding (8 cores): core i handles capsule n = i//2 and batch half h = i%2
(512 batch rows), processed in 4 chunks of 128 rows (partition width).

Per-core dataflow (per 128-row chunk):
  stage A : u[b,s,c] = (mask*behaviors)[b,s,:] @ W[n,s]  -- 2 matmuls per s
            (one into a rotating 8-s PSUM group for u, one accumulating
            z0 = sum_s u into a fixed PSUM bank); ACT evacuates u to SBUF
            as bf16 [P, S, C].  W is SBUF-resident (loaded once per core).
  r0      : caps0 = squash(z0 * invn); delta0 = sum_c u*caps0 via one big
            bf16 TT mult (2x mode) + pairwise tree over c per 50-s quarter;
            logits = delta0 + mneg.
  r1/r2   : softmax: mx, small ACT exp (for Z), lmR = (logits-mx) stored as
            duplicated bf16 R-tuples [P,S,R]; coup expansion to [P,50,C]
            ON THE SCALAR ENGINE with a broadcast AP (reads [v|v] 32-bit
            pairs -> packed mode), then DVE in-place mult with u + pairwise
            tree over s -> zraw; caps = squash(zraw * invZ).
            r1 additionally computes delta1 (like delta0) into logits.

Chunks are software-pipelined: stage A of chunk k+1 is emitted after the
routing of chunk k, so the Tile priority scheduler overlaps PE/ACT work of
the next chunk with DVE routing of the current one; routing's scalar-engine
ops carry explicit high priority so evacuations don't preempt them.

kernel() takes FULL inputs and returns the FULL [1024, 4, 128] fp32 output.
"""

import numpy as np
import ml_dtypes

BF16 = ml_dtypes.bfloat16

B, S, E, C, NCAPS = 1024, 200, 128, 128, 4
NCORES = 8
BH = B // 2          # batch rows per core
P = 128              # partition width / chunk size
NCHUNK = BH // P     # 4 chunks per core
SBLK = 25            # s per DMA block
NBLK = S // SBLK     # 8 blocks
QS = 50              # s per routing quarter
NQ = S // QS         # 4 quarters
GS = 8               # s per PSUM evac group
NEG = -3.0e38
EXPR = 2             # duplication factor for the exp-expand input

_COMPILED = {}


def _emit(ctx, tc, nc):
    import concourse.bass as bass
    from concourse import mybir

    bf = mybir.dt.bfloat16
    f32 = mybir.dt.float32

    bmt = nc.dram_tensor("bmt", [E, NCHUNK, S, P], bf, kind="ExternalInput").ap()
    wt = nc.dram_tensor("wt", [E, S, C], bf, kind="ExternalInput").ap()
    mneg = nc.dram_tensor("mneg", [NCHUNK, P, S], bf, kind="ExternalInput").ap()
    invn = nc.dram_tensor("invn", [NCHUNK, P, 1], f32, kind="ExternalInput").ap()
    out = nc.dram_tensor("caps_out", [NCHUNK, P, C], f32, kind="ExternalOutput").ap()

    wtpool = ctx.enter_context(tc.tile_pool(name="wt", bufs=NBLK))
    bmpool = ctx.enter_context(tc.tile_pool(name="bm", bufs=2))
    upool = ctx.enter_context(tc.tile_pool(name="u", bufs=2))
    scpool = ctx.enter_context(tc.tile_pool(name="scratch", bufs=2))
    smalls = ctx.enter_context(tc.tile_pool(name="smalls", bufs=3))
    pupool = ctx.enter_context(tc.tile_pool(name="pu", bufs=3, space="PSUM"))
    pzpool = ctx.enter_context(tc.tile_pool(name="pz", bufs=2, space="PSUM"))

    # W resident in SBUF: 8 tiles of [E, 25, C], loaded lazily inside
    # stage_a(0) interleaved with the bt blocks so block 0's operands land
    # on parallel DMA lanes first (minimizes the pipeline-fill latency).
    wt_tiles = []

    def squash_scale(zraw, invz, capsf, in_sbuf=True):
        # capsf = alpha' * zraw with alpha' = n2t/((1+n2t)*sqrt(n2)),
        # n2 = sum(zraw^2), n2t = n2*invz^2.  Uses sqrt(n2t) = sqrt(n2)*invz
        # so the ACT sqrt runs in parallel with the DVE scale.  capsf dtype
        # picks the consumer: bf16 for the routing mults (r0/r1), f32 for
        # the r2 output.  n2 is computed on DVE when zraw is in SBUF (no
        # ACT round-trip on the critical path); the PSUM-resident r0 case
        # uses the ACT Square-accumulate (ACT reads PSUM natively).
        n2 = smalls.tile([P, 1], f32, tag="n2")
        if in_sbuf:
            sqt = smalls.tile([P, C], f32, tag="sqt")
            nc.vector.tensor_mul(sqt, zraw, zraw)
            nc.vector.tensor_reduce(
                out=n2, in_=sqt, axis=mybir.AxisListType.X,
                op=mybir.AluOpType.add,
            )
        else:
            sqt = smalls.tile([P, C], bf, tag="sqtb")
            with tc.high_priority():
                nc.scalar.activation(
                    out=sqt, in_=zraw,
                    func=mybir.ActivationFunctionType.Square,
                    accum_out=n2,
                )
        sq2 = smalls.tile([P, 1], f32, tag="st")
        with tc.high_priority():
            nc.scalar.sqrt(sq2, n2)
        n2t = smalls.tile([P, 1], f32, tag="n2t")
        nc.vector.tensor_scalar(
            out=n2t, in0=n2, scalar1=invz, scalar2=invz,
            op0=mybir.AluOpType.mult, op1=mybir.AluOpType.mult,
        )
        den = smalls.tile([P, 1], f32, tag="den")
        nc.vector.scalar_tensor_tensor(
            out=den, in0=n2t, scalar=1.0, in1=sq2,
            op0=mybir.AluOpType.add, op1=mybir.AluOpType.mult,
        )
        rd = smalls.tile([P, 1], f32, tag="rd")
        nc.vector.reciprocal(rd, den)
        alphap = smalls.tile([P, 1], f32, tag="alphap")
        nc.vector.tensor_mul(alphap, n2t, rd)
        with tc.high_priority():
            nc.scalar.mul(capsf, zraw, alphap)

    def tree_c(sc, dst):
        # sc: [P, QS, C] bf16 reduced in place over c (C power of 2) down to
        # width 8, then one 1x-mode tensor_reduce finishes into f32 dst.
        w = C
        while w > 8:
            h = w // 2
            nc.vector.tensor_add(sc[:, :, 0:h], sc[:, :, 0:h], sc[:, :, h:w])
            w = h
        nc.vector.tensor_reduce(
            out=dst, in_=sc[:, :, 0:8], axis=mybir.AxisListType.X,
            op=mybir.AluOpType.add,
        )

    def tree_s(sc, dst):
        # sc: [P, QS, C] bf16 reduced in place over s; dst: [P, C] f32.
        w = QS
        while w > 2:
            h = w // 2
            nxt = (w + 1) // 2
            nc.vector.tensor_add(sc[:, 0:h, :], sc[:, 0:h, :], sc[:, nxt:nxt + h, :])
            w = nxt
        nc.vector.tensor_add(dst, sc[:, 0, :], sc[:, 1, :])

    chunk_state = {}

    def stage_a(k):
        u = upool.tile([P, S, C], bf, tag="u")
        pz = pzpool.tile([P, C], f32, tag="pz")
        pu = None
        for blk in range(NBLK):
            bt = bmpool.tile([E, SBLK, P], bf, tag="bt")
            if k == 0:
                # 4-way split across DMA lanes, bt pieces ahead of wt
                for p0, p1 in ((0, 7), (7, 13), (13, 19), (19, 25)):
                    nc.sync.dma_start(
                        out=bt[:, p0:p1, :],
                        in_=bmt[:, k, blk * SBLK + p0:blk * SBLK + p1, :])
                t = wtpool.tile([E, SBLK, C], bf, tag="wt")
                for p0, p1 in ((0, 7), (7, 13), (13, 19), (19, 25)):
                    nc.sync.dma_start(
                        out=t[:, p0:p1, :],
                        in_=wt[:, blk * SBLK + p0:blk * SBLK + p1, :])
                wt_tiles.append(t)
            else:
                for p0, p1 in ((0, 13), (13, 25)):
                    nc.sync.dma_start(
                        out=bt[:, p0:p1, :],
                        in_=bmt[:, k, blk * SBLK + p0:blk * SBLK + p1, :])
            for j in range(SBLK):
                s = blk * SBLK + j
                q = s % GS
                if q == 0:
                    pu = pupool.tile([P, GS * C], f32, tag="pu")
                nc.tensor.matmul(
                    pu[:, q * C:(q + 1) * C], lhsT=bt[:, j, :],
                    rhs=wt_tiles[blk][:, j, :], start=True, stop=True,
                )
                nc.tensor.matmul(
                    pz, lhsT=bt[:, j, :], rhs=wt_tiles[blk][:, j, :],
                    start=(s == 0), stop=(s == S - 1), skip_group_check=True,
                )
                if q == GS - 1:
                    pv = pu.rearrange("p (g c) -> p g c", c=C)
                    nc.scalar.copy(u[:, s - GS + 1:s + 1, :], pv)
        mneg_sb = smalls.tile([P, S], bf, tag="mneg")
        nc.sync.dma_start(out=mneg_sb, in_=mneg[k])
        invn_sb = smalls.tile([P, 1], f32, tag="invn")
        nc.sync.dma_start(out=invn_sb, in_=invn[k])
        chunk_state[k] = (u, pz, mneg_sb, invn_sb)

    def routing(k):
        u, pz, mneg_sb, invn_sb = chunk_state.pop(k)

        logits = smalls.tile([P, S], f32, tag="logits")
        caps = smalls.tile([P, C], f32, tag="caps")
        capsb = smalls.tile([P, C], bf, tag="capsb")
        zraw = smalls.tile([P, C], f32, tag="zraw")
        zq = smalls.tile([P, C], f32, tag="zq")
        invz = smalls.tile([P, 1], f32, tag="invz")

        # ---------- r0 ----------  (squash reads z0 straight from PSUM)
        squash_scale(pz, invn_sb, capsb, in_sbuf=False)

        # delta0 -> logits
        for qd in range(NQ):
            sl = slice(qd * QS, (qd + 1) * QS)
            sc = scpool.tile([P, QS, C], bf, tag="sc")
            capse = capsb.unsqueeze(1).broadcast_to([P, QS, C])
            nc.vector.tensor_mul(sc, u[:, sl, :], capse)
            tree_c(sc, logits[:, sl])
        nc.vector.tensor_add(logits, logits, mneg_sb)

        for r in (1, 2):
            # softmax pieces
            mx = smalls.tile([P, 1], f32, tag="mx")
            nc.vector.tensor_reduce(
                out=mx, in_=logits, axis=mybir.AxisListType.X,
                op=mybir.AluOpType.max,
            )
            negmx = smalls.tile([P, 1], f32, tag="negmx")
            nc.vector.tensor_scalar_mul(negmx, mx, -1.0)
            zsum = smalls.tile([P, 1], f32, tag="zsum")
            ejunk = smalls.tile([P, S], bf, tag="ejunk")
            with tc.high_priority():
                nc.scalar.activation(
                    out=ejunk, in_=logits,
                    func=mybir.ActivationFunctionType.Exp,
                    bias=negmx, scale=1.0, accum_out=zsum,
                )
            nc.vector.reciprocal(invz, zsum)
            # lmR[p, s, t] = (logits - mx) duplicated EXPR times, bf16
            lmr = smalls.tile([P, S, EXPR], bf, tag="lmr")
            lsrc = logits.unsqueeze(2).broadcast_to([P, S, EXPR])
            nc.vector.tensor_scalar(
                out=lmr, in0=lsrc, scalar1=negmx, scalar2=0.0,
                op0=mybir.AluOpType.add, op1=mybir.AluOpType.add,
            )

            zsum = smalls.tile([P, 1], f32, tag="zsum")
            ejunk = smalls.tile([P, S], bf, tag="ejunk")
            nc.scalar.activation(
                out=ejunk, in_=logits,
                func=mybir.ActivationFunctionType.Exp,
                bias=negmx, scale=1.0, accum_out=zsum,
            )
            nc.vector.reciprocal(invz, zsum)

            # zraw = sum_s exp(lm)*u
            for qz in range(NQ):
                sl = slice(qz * QS, (qz + 1) * QS)
                sc = scpool.tile([P, QS, C], bf, tag="sc")
                if qz == 0:
                    pieces = ((0, 13), (13, 25), (25, 50))
                elif qz == 1:
                    pieces = ((0, 25), (25, 50))
                else:
                    pieces = ((0, 50),)
                for h0, h1 in pieces:
                    hsl = slice(qz * QS + h0, qz * QS + h1)
                    scs = sc[:, h0:h1, :]
                    hw_ = h1 - h0
                    lmv = lmr[:, hsl, :].unsqueeze(2).broadcast_to(
                        [P, hw_, C // EXPR, EXPR])
                    scv = scs.rearrange("p s (h t) -> p s h t", t=EXPR)
                    with tc.high_priority():
                        nc.scalar.activation(
                            out=scv, in_=lmv,
                            func=mybir.ActivationFunctionType.Exp,
                        )
                    nc.vector.tensor_mul(scs, scs, u[:, hsl, :])
                if qz == 0:
                    tree_s(sc, zraw)
                else:
                    tree_s(sc, zq)
                    nc.vector.tensor_add(zraw, zraw, zq)

            squash_scale(zraw, invz, caps if r == 2 else capsb)

            if r == 1:
                # delta1 -> logits +=
                for qd in range(NQ):
                    sl = slice(qd * QS, (qd + 1) * QS)
                    sc = scpool.tile([P, QS, C], bf, tag="sc")
                    capse = capsb.unsqueeze(1).broadcast_to([P, QS, C])
                    nc.vector.tensor_mul(sc, u[:, sl, :], capse)
                    dl = smalls.tile([P, QS], f32, tag="dl")
                    tree_c(sc, dl)
                    nc.vector.tensor_add(logits[:, sl], logits[:, sl], dl)

        nc.sync.dma_start(out=out[k], in_=caps)

    # Emit stage_a(k) then routing(k): the scheduler overlaps the next
    # chunk's stage A (emitted later, lower priority) with routing(k),
    # and routing's ACT ops carry explicit high priority so evacuations
    # don't preempt them.
    for k in range(NCHUNK):
        stage_a(k)
        routing(k)


def _build():
    if "nc" in _COMPILED:
        return _COMPILED["nc"]
    from contextlib import ExitStack
    import concourse.bacc as bacc
    import concourse.tile as tile

    nc = bacc.Bacc(
        "TRN2", target_bir_lowering=False, debug=False, enable_asserts=False
    )
    with tile.TileContext(nc, trace_sim=False) as tc, ExitStack() as ctx:
        _emit(ctx, tc, nc)
    nc.compile()
    _COMPILED["nc"] = nc
    return nc


def make_in_maps(behaviors, valid_mask, W):
    behaviors = np.asarray(behaviors, dtype=np.float32)
    mask = np.asarray(valid_mask).astype(bool)
    W = np.asarray(W, dtype=np.float32)

    bm = behaviors * mask[:, :, None].astype(np.float32)          # [B,S,E]
    mneg_full = np.where(mask, 0.0, NEG).astype(BF16)             # [B,S]
    nval = mask.sum(axis=1).astype(np.float32)
    invn_full = (1.0 / np.maximum(nval, 1.0)).astype(np.float32)  # [B]

    in_maps = []
    for core in range(NCORES):
        n, h = core // 2, core % 2
        bsl = slice(h * BH, (h + 1) * BH)
        bmh = bm[bsl].reshape(NCHUNK, P, S, E)
        # bmt[e, k, s, p] = bmh[k, p, s, e]
        bmt_core = np.ascontiguousarray(bmh.transpose(3, 0, 2, 1)).astype(BF16)
        wt_core = np.ascontiguousarray(W[n].transpose(1, 0, 2)).astype(BF16)
        in_maps.append({
            "bmt": bmt_core,
            "wt": wt_core,
            "mneg": np.ascontiguousarray(mneg_full[bsl].reshape(NCHUNK, P, S)),
            "invn": np.ascontiguousarray(invn_full[bsl].reshape(NCHUNK, P, 1)),
        })
    return in_maps


def gather_output(results):
    out = np.empty((B, NCAPS, C), dtype=np.float32)
    for core in range(NCORES):
        n, h = core // 2, core % 2
        caps = results[core]["caps_out"].reshape(BH, C)
        out[h * BH:(h + 1) * BH, n, :] = caps
    return out


def kernel(behaviors, valid_mask, W):
    from concourse import bass_utils

    nc = _build()
    in_maps = make_in_maps(behaviors, valid_mask, W)
    res = bass_utils.run_bass_kernel_spmd(nc, in_maps, core_ids=list(range(NCORES)))
    return gather_output(res.results)
